# revision 39
# baseline (speedup 1.0000x reference)
"""Trainium2 Bass kernel for the interval-prediction custom loss.

total = 10*mean((t - (l+u)/2)^2) + 0.1*mean(u-l) + 10*mean(relu(l-u))
        + 0.5*sum(where(pv==0, relu(c-p), relu(p-c)))/N        with c=(l+u)/2

Strategy: pure data parallel over N across 8 NeuronCores; host does only the
tiny final scalar reduction in float64.

Engine plan (v10). Measured facts from earlier traces: DVE tensor_tensor
runs 2x and tensor_scalar 4x at bf16, but ANY DVE op with an accumulator
drops to 1x; GPSIMD elementwise ops steal the DVE's SBUF port; heavy PE
matmul traffic slows the DMA streams (SBUF bandwidth contention); ACT costs
~(fd+352)/1.2 ns per pass regardless of dtype.

The pv indicator is encoded host-side as s = +-1 (same boolean, exact in
bf16), so the direction penalty is a single reduction:
  where(pv==0, relu(x), relu(-x)) = relu(s*x).

  DVE:    H = lo + up            (TT 2x)   [tile j]
          w = lo - up            (TT 2x)   [tile j]
          c = 0.5*H              (TS 4x)   [tile j]
          x = c - p              (TT 2x)   [tile j-1, staggered so the
          e = c - t              (TT 2x)    second half never waits on
          sx = s * x             (TT 2x)    the t/p/s DMA]
          rsx = max(sx, 0)       (TS 4x)
  ACT:    Relu(w)    accum -> S_vd    (= sum relu(lo-up))
          Square(e)  accum -> S_sq
  PE:     ones^T * w   -> PSUM[1,512]   (S_w;  width sum = -S_w)
          ones^T * rsx -> PSUM[1,512]   (S_d = direction penalty)

All five streams are bf16; all accumulation is fp32.  Tile widths are
multiples of 512 so each PE matmul chunk is exactly 512 (one PSUM bank).
[lo,up] is a separate SBUF tile from [t,p,s] so H/w start as soon as the
first DMA lands; io pools are deep enough that no dma_start ever waits on a
tile free (head-of-line blocking on the sync sequencer stalls all 16 DMA
queues).
"""

import sys

if "/opt/trn_rl_repo" not in sys.path:
    sys.path.insert(0, "/opt/trn_rl_repo")

import numpy as np

N = 8388608
N_CORES = 8
P = 128
NP_PER_CORE = N // N_CORES            # 1048576
FPL = NP_PER_CORE // P                # 8192 elements per partition lane
TILE_WIDTHS = (512, 2048, 2048, 2048, 1024, 512)
assert sum(TILE_WIDTHS) == FPL
assert all(w % 512 == 0 for w in TILE_WIDTHS)
MM = 512                              # matmul moving chunk / PSUM bank width

_NC_CACHE = {}


def _build(fpl=FPL, widths=TILE_WIDTHS):
    """Build the per-core Bass program (identical on all cores)."""
    from concourse import bacc, mybir
    from concourse.tile import TileContext

    assert sum(widths) == fpl
    n_tiles = len(widths)

    f32 = mybir.dt.float32
    bf16 = mybir.dt.bfloat16
    Alu = mybir.AluOpType
    Act = mybir.ActivationFunctionType

    nc = bacc.Bacc(trn_type="TRN2")
    big = nc.declare_dram_parameter("big", [P, 5 * fpl], bf16, isOutput=False)
    consts = nc.declare_dram_parameter("consts", [P, 1], bf16, isOutput=False)
    # accumulator columns: S_sq (n_tiles) | S_vd (n_tiles)
    out = nc.declare_dram_parameter("out", [P, 2 * n_tiles], f32, isOutput=True)
    # rows: S_w | S_d column sums
    sums = nc.declare_dram_parameter("sums", [2, MM], f32, isOutput=True)

    with TileContext(nc) as tc:
        with (
            tc.tile_pool(name="ioa", bufs=6) as ioa_pool,
            tc.tile_pool(name="iob", bufs=6) as iob_pool,
            tc.tile_pool(name="mid", bufs=3) as mid_pool,
            tc.tile_pool(name="sht", bufs=2) as sht_pool,
            tc.tile_pool(name="jnk", bufs=1) as jnk_pool,
            tc.tile_pool(name="acc", bufs=1) as acc_pool,
            tc.tile_pool(name="pss", bufs=1, space="PSUM") as pss_pool,
        ):
            acc_act = acc_pool.tile([P, 2 * n_tiles], f32, tag="acc_act")
            sw_sb = acc_pool.tile([1, MM], f32, tag="sw_sb")
            sd_sb = acc_pool.tile([1, MM], f32, tag="sd_sb")
            ps_w = pss_pool.tile([1, MM], f32, tag="ps_w")
            ps_d = pss_pool.tile([1, MM], f32, tag="ps_d")

            const_t = acc_pool.tile([P, 1], bf16, tag="consts")
            onesv = const_t[:, 0:1]

            first = [True, True]
            n_chunks = fpl // MM

            off = 0
            done_chunks = 0
            for j, tw in enumerate(widths):
                big_a = ioa_pool.tile([P, 2, tw], bf16, tag="biga", name=f"biga{j}")
                big_b = iob_pool.tile([P, 3, tw], bf16, tag="bigb", name=f"bigb{j}")
                src = big[:, off : off + 5 * tw].rearrange("p (s f) -> p s f", s=5)
                nc.sync.dma_start(out=big_a, in_=src[:, 0:2, :])
                if j == 0:
                    # consts needed only by the PE; issue after the first
                    # compute-critical DMA
                    nc.sync.dma_start(out=const_t, in_=consts[:, :])
                # issue the [t,p,s] DMA from the second HWDGE engine so the
                # two issue queues run in parallel
                nc.scalar.dma_start(out=big_b, in_=src[:, 2:5, :])
                off += 5 * tw

                lo = big_a[:, 0, :]
                up = big_a[:, 1, :]
                t_t = big_b[:, 0, :]
                p_t = big_b[:, 1, :]
                s_t = big_b[:, 2, :]

                H = sht_pool.tile([P, tw], bf16, tag="H", name=f"H{j}")
                w = mid_pool.tile([P, tw], bf16, tag="w", name=f"w{j}")
                c = sht_pool.tile([P, tw], bf16, tag="c", name=f"c{j}")
                x = sht_pool.tile([P, tw], bf16, tag="x", name=f"x{j}")
                e = mid_pool.tile([P, tw], bf16, tag="e", name=f"e{j}")
                sx = sht_pool.tile([P, tw], bf16, tag="sx", name=f"sx{j}")
                rsx = mid_pool.tile([P, tw], bf16, tag="rsx", name=f"rsx{j}")
                ja = jnk_pool.tile([P, tw], bf16, tag="ja", name=f"ja{j}")
                jd = jnk_pool.tile([P, tw], bf16, tag="jd", name=f"jd{j}")

                # --- DVE: fast-mode ops only, no accumulators ---
                nc.vector.tensor_add(out=H, in0=lo, in1=up)
                nc.vector.tensor_sub(out=w, in0=lo, in1=up)
                nc.vector.tensor_scalar(
                    out=c, in0=H, scalar1=0.5, scalar2=None, op0=Alu.mult
                )
                nc.vector.tensor_sub(out=x, in0=c, in1=p_t)
                nc.vector.tensor_sub(out=e, in0=c, in1=t_t)
                nc.vector.tensor_mul(out=sx, in0=s_t, in1=x)
                nc.vector.tensor_scalar(
                    out=rsx, in0=sx, scalar1=0.0, scalar2=None, op0=Alu.max
                )

                # --- ACT: the two nonlinear accumulations ---
                nc.scalar.activation(
                    out=jd, in_=w, func=Act.Relu,
                    accum_out=acc_act[:, n_tiles + j : n_tiles + j + 1],
                )
                nc.scalar.activation(
                    out=ja, in_=e, func=Act.Square,
                    accum_out=acc_act[:, j : j + 1],
                )

                # --- PE: column-sum matmuls (single ones stationary) ---
                for ci, ch in enumerate(range(0, tw, MM)):
                    is_last = done_chunks + ci == n_chunks - 1
                    nc.tensor.matmul(
                        ps_w, onesv, w[:, ch : ch + MM],
                        start=first[0], stop=is_last,
                    )
                    first[0] = False
                for ci, ch in enumerate(range(0, tw, MM)):
                    is_last = done_chunks + ci == n_chunks - 1
                    nc.tensor.matmul(
                        ps_d, onesv, rsx[:, ch : ch + MM],
                        start=first[1], stop=is_last,
                    )
                    first[1] = False
                done_chunks += tw // MM

            # PSUM -> SBUF -> DRAM for the column sums
            nc.scalar.activation(out=sw_sb[:, :], in_=ps_w, func=Act.Copy)
            nc.scalar.activation(out=sd_sb[:, :], in_=ps_d, func=Act.Copy)

            nc.sync.dma_start(out=out[:, :], in_=acc_act)
            nc.sync.dma_start(out=sums[0:1, :], in_=sw_sb)
            nc.sync.dma_start(out=sums[1:2, :], in_=sd_sb)

    nc.compile()
    return nc


def _get_nc():
    key = (FPL, TILE_WIDTHS)
    if key not in _NC_CACHE:
        _NC_CACHE[key] = _build()
    return _NC_CACHE[key]


def _make_consts():
    import ml_dtypes

    return np.ones((P, 1), dtype=ml_dtypes.bfloat16)


def _shard(inputs, fpl=FPL, widths=TILE_WIDTHS, n_cores=N_CORES):
    import ml_dtypes

    bf = ml_dtypes.bfloat16
    n = n_cores * P * fpl
    pred = np.asarray(inputs["pred"])
    targ = np.asarray(inputs["target"]).reshape(n)
    prev = np.asarray(inputs["prev_pci"]).reshape(n)
    # pv is a 0/1 indicator; encode it as +-1 (exact in bf16):
    # where(pv==0, relu(x), relu(-x)) = relu(s*x) with s = 1-2*pv.
    pv = np.asarray(inputs["pv_values"])
    sgn = np.where(pv == 0, bf(1.0), bf(-1.0)).astype(bf).reshape(n)

    lo = pred[:, 0].astype(bf)
    up = pred[:, 1].astype(bf)
    tb = targ.astype(bf)
    pb = prev.astype(bf)

    consts = _make_consts()
    np_per_core = P * fpl

    in_maps = []
    for cix in range(n_cores):
        s = slice(cix * np_per_core, (cix + 1) * np_per_core)
        streams = (
            lo[s].reshape(P, fpl),
            up[s].reshape(P, fpl),
            tb[s].reshape(P, fpl),
            pb[s].reshape(P, fpl),
            sgn[s].reshape(P, fpl),
        )
        # tile-major: per partition, each tile's 5 stream-chunks contiguous
        parts = []
        off = 0
        for fd in widths:
            for st in streams:
                parts.append(st[:, off : off + fd])
            off += fd
        big = np.concatenate(parts, axis=1)
        in_maps.append({"big": np.ascontiguousarray(big), "consts": consts})
    return in_maps


def _combine(core_outs, core_sums, widths=TILE_WIDTHS, n=N):
    """core_outs: [P, 2*n_tiles] ACT accumulators per core (S_sq | S_vd).
    core_sums: [2, MM] column sums per core (S_w | S_d)."""
    n_tiles = len(widths)
    s_sq = s_vd = s_w = s_d = 0.0
    for o, ss in zip(core_outs, core_sums):
        o = np.asarray(o, dtype=np.float64)
        ss = np.asarray(ss, dtype=np.float64)
        s_sq += o[:, 0:n_tiles].sum()
        s_vd += o[:, n_tiles : 2 * n_tiles].sum()
        s_w += ss[0].sum()
        s_d += ss[1].sum()

    center_loss = s_sq / n
    width_loss = -s_w / n                  # sum(up - lo) = -sum(lo - up)
    valid_penalty = s_vd / n
    direction_penalty = s_d
    total = (
        center_loss * 10.0
        + 0.1 * width_loss
        + 10.0 * valid_penalty
        + 0.5 * direction_penalty / n
    )
    return np.float32(total)


def _run(inputs, trace=False):
    """Run the SPMD kernel; returns (scalar_result, BassKernelResults)."""
    from concourse.bass_utils import run_bass_kernel_spmd

    nc = _get_nc()
    in_maps = _shard(inputs)
    res = run_bass_kernel_spmd(
        nc, in_maps, core_ids=list(range(N_CORES)), trace=trace
    )
    core_outs = [res.results[c]["out"] for c in range(N_CORES)]
    core_sums = [res.results[c]["sums"] for c in range(N_CORES)]
    return _combine(core_outs, core_sums), res


def kernel(**inputs) -> np.ndarray:
    result, _ = _run(inputs, trace=False)
    return result


# revision 40
# speedup vs baseline: 1.1139x; 1.1139x over previous
"""Trainium2 Bass kernel for the interval-prediction custom loss.

total = 10*mean((t - (l+u)/2)^2) + 0.1*mean(u-l) + 10*mean(relu(l-u))
        + 0.5*sum(where(pv==0, relu(c-p), relu(p-c)))/N        with c=(l+u)/2

Strategy: pure data parallel over N across 8 NeuronCores; host does only the
tiny final scalar reduction in float64.

Engine plan (v10). Measured facts from earlier traces: DVE tensor_tensor
runs 2x and tensor_scalar 4x at bf16, but ANY DVE op with an accumulator
drops to 1x; GPSIMD elementwise ops steal the DVE's SBUF port; heavy PE
matmul traffic slows the DMA streams (SBUF bandwidth contention); ACT costs
~(fd+352)/1.2 ns per pass regardless of dtype.

The pv indicator is encoded host-side as s = +-1 (same boolean, exact in
bf16), so the direction penalty is a single reduction:
  where(pv==0, relu(x), relu(-x)) = relu(s*x).

  DVE:    H = lo + up            (TT 2x)   [tile j]
          w = lo - up            (TT 2x)   [tile j]
          c = 0.5*H              (TS 4x)   [tile j]
          x = c - p              (TT 2x)   [tile j-1, staggered so the
          e = c - t              (TT 2x)    second half never waits on
          sx = s * x             (TT 2x)    the t/p/s DMA]
          rsx = max(sx, 0)       (TS 4x)
  ACT:    Relu(w)    accum -> S_vd    (= sum relu(lo-up))
          Square(e)  accum -> S_sq
  PE:     ones^T * w   -> PSUM[1,512]   (S_w;  width sum = -S_w)
          ones^T * rsx -> PSUM[1,512]   (S_d = direction penalty)

All five streams are bf16; all accumulation is fp32.  Tile widths are
multiples of 512 so each PE matmul chunk is exactly 512 (one PSUM bank).
[lo,up] is a separate SBUF tile from [t,p,s] so H/w start as soon as the
first DMA lands; io pools are deep enough that no dma_start ever waits on a
tile free (head-of-line blocking on the sync sequencer stalls all 16 DMA
queues).
"""

import sys

if "/opt/trn_rl_repo" not in sys.path:
    sys.path.insert(0, "/opt/trn_rl_repo")

import numpy as np

N = 8388608
N_CORES = 8
P = 128
NP_PER_CORE = N // N_CORES            # 1048576
FPL = NP_PER_CORE // P                # 8192 elements per partition lane
TILE_WIDTHS = (512, 2048, 2048, 2048, 1024, 512)
assert sum(TILE_WIDTHS) == FPL
assert all(w % 512 == 0 for w in TILE_WIDTHS)
MM = 512                              # matmul moving chunk / PSUM bank width

_NC_CACHE = {}


def _build(fpl=FPL, widths=TILE_WIDTHS):
    """Build the per-core Bass program (identical on all cores)."""
    from concourse import bacc, mybir
    from concourse.tile import TileContext

    assert sum(widths) == fpl
    n_tiles = len(widths)

    f32 = mybir.dt.float32
    bf16 = mybir.dt.bfloat16
    Alu = mybir.AluOpType
    Act = mybir.ActivationFunctionType

    nc = bacc.Bacc(trn_type="TRN2")
    big = nc.declare_dram_parameter("big", [P, 5 * fpl], bf16, isOutput=False)
    consts = nc.declare_dram_parameter("consts", [P, 1], bf16, isOutput=False)
    # accumulator columns: S_sq (n_tiles) | S_vd (n_tiles)
    out = nc.declare_dram_parameter("out", [P, 2 * n_tiles], f32, isOutput=True)
    # rows: S_w | S_d column sums
    sums = nc.declare_dram_parameter("sums", [2, MM], f32, isOutput=True)

    with TileContext(nc) as tc:
        with (
            tc.tile_pool(name="ioa", bufs=6) as ioa_pool,
            tc.tile_pool(name="iob", bufs=6) as iob_pool,
            tc.tile_pool(name="mid", bufs=3) as mid_pool,
            tc.tile_pool(name="sht", bufs=2) as sht_pool,
            tc.tile_pool(name="jnk", bufs=1) as jnk_pool,
            tc.tile_pool(name="acc", bufs=1) as acc_pool,
            tc.tile_pool(name="pss", bufs=1, space="PSUM") as pss_pool,
        ):
            acc_act = acc_pool.tile([P, 2 * n_tiles], f32, tag="acc_act")
            sw_sb = acc_pool.tile([1, MM], f32, tag="sw_sb")
            sd_sb = acc_pool.tile([1, MM], f32, tag="sd_sb")
            ps_w = pss_pool.tile([1, MM], f32, tag="ps_w")
            ps_d = pss_pool.tile([1, MM], f32, tag="ps_d")

            const_t = acc_pool.tile([P, 1], bf16, tag="consts")
            onesv = const_t[:, 0:1]

            first = [True, True]
            n_chunks = fpl // MM

            off = 0
            done_chunks = 0
            for j, tw in enumerate(widths):
                big_a = ioa_pool.tile([P, 2, tw], bf16, tag="biga", name=f"biga{j}")
                big_b = iob_pool.tile([P, 3, tw], bf16, tag="bigb", name=f"bigb{j}")
                src = big[:, off : off + 5 * tw].rearrange("p (s f) -> p s f", s=5)
                nc.sync.dma_start(out=big_a, in_=src[:, 0:2, :])
                if j == 0:
                    # consts needed only by the PE; issue after the first
                    # compute-critical DMA
                    nc.sync.dma_start(out=const_t, in_=consts[:, :])
                nc.sync.dma_start(out=big_b, in_=src[:, 2:5, :])
                off += 5 * tw

                lo = big_a[:, 0, :]
                up = big_a[:, 1, :]
                t_t = big_b[:, 0, :]
                p_t = big_b[:, 1, :]
                s_t = big_b[:, 2, :]

                H = sht_pool.tile([P, tw], bf16, tag="H", name=f"H{j}")
                w = mid_pool.tile([P, tw], bf16, tag="w", name=f"w{j}")
                c = sht_pool.tile([P, tw], bf16, tag="c", name=f"c{j}")
                x = sht_pool.tile([P, tw], bf16, tag="x", name=f"x{j}")
                e = mid_pool.tile([P, tw], bf16, tag="e", name=f"e{j}")
                sx = sht_pool.tile([P, tw], bf16, tag="sx", name=f"sx{j}")
                rsx = mid_pool.tile([P, tw], bf16, tag="rsx", name=f"rsx{j}")
                ja = jnk_pool.tile([P, tw], bf16, tag="ja", name=f"ja{j}")
                jd = jnk_pool.tile([P, tw], bf16, tag="jd", name=f"jd{j}")

                # --- DVE: fast-mode ops only, no accumulators ---
                nc.vector.tensor_add(out=H, in0=lo, in1=up)
                nc.vector.tensor_sub(out=w, in0=lo, in1=up)
                nc.vector.tensor_scalar(
                    out=c, in0=H, scalar1=0.5, scalar2=None, op0=Alu.mult
                )
                nc.vector.tensor_sub(out=x, in0=c, in1=p_t)
                nc.vector.tensor_sub(out=e, in0=c, in1=t_t)
                nc.vector.tensor_mul(out=sx, in0=s_t, in1=x)
                nc.vector.tensor_scalar(
                    out=rsx, in0=sx, scalar1=0.0, scalar2=None, op0=Alu.max
                )

                # --- ACT: the two nonlinear accumulations ---
                nc.scalar.activation(
                    out=jd, in_=w, func=Act.Relu,
                    accum_out=acc_act[:, n_tiles + j : n_tiles + j + 1],
                )
                nc.scalar.activation(
                    out=ja, in_=e, func=Act.Square,
                    accum_out=acc_act[:, j : j + 1],
                )

                # --- PE: column-sum matmuls (single ones stationary) ---
                for ci, ch in enumerate(range(0, tw, MM)):
                    is_last = done_chunks + ci == n_chunks - 1
                    nc.tensor.matmul(
                        ps_w, onesv, w[:, ch : ch + MM],
                        start=first[0], stop=is_last,
                    )
                    first[0] = False
                for ci, ch in enumerate(range(0, tw, MM)):
                    is_last = done_chunks + ci == n_chunks - 1
                    nc.tensor.matmul(
                        ps_d, onesv, rsx[:, ch : ch + MM],
                        start=first[1], stop=is_last,
                    )
                    first[1] = False
                done_chunks += tw // MM

            # PSUM -> SBUF -> DRAM for the column sums
            nc.scalar.activation(out=sw_sb[:, :], in_=ps_w, func=Act.Copy)
            nc.scalar.activation(out=sd_sb[:, :], in_=ps_d, func=Act.Copy)

            nc.sync.dma_start(out=out[:, :], in_=acc_act)
            nc.sync.dma_start(out=sums[0:1, :], in_=sw_sb)
            nc.sync.dma_start(out=sums[1:2, :], in_=sd_sb)

    nc.compile()
    return nc


def _get_nc():
    key = (FPL, TILE_WIDTHS)
    if key not in _NC_CACHE:
        _NC_CACHE[key] = _build()
    return _NC_CACHE[key]


def _make_consts():
    import ml_dtypes

    return np.ones((P, 1), dtype=ml_dtypes.bfloat16)


def _shard(inputs, fpl=FPL, widths=TILE_WIDTHS, n_cores=N_CORES):
    import ml_dtypes

    bf = ml_dtypes.bfloat16
    n = n_cores * P * fpl
    pred = np.asarray(inputs["pred"])
    targ = np.asarray(inputs["target"]).reshape(n)
    prev = np.asarray(inputs["prev_pci"]).reshape(n)
    # pv is a 0/1 indicator; encode it as +-1 (exact in bf16):
    # where(pv==0, relu(x), relu(-x)) = relu(s*x) with s = 1-2*pv.
    pv = np.asarray(inputs["pv_values"])
    sgn = np.where(pv == 0, bf(1.0), bf(-1.0)).astype(bf).reshape(n)

    lo = pred[:, 0].astype(bf)
    up = pred[:, 1].astype(bf)
    tb = targ.astype(bf)
    pb = prev.astype(bf)

    consts = _make_consts()
    np_per_core = P * fpl

    in_maps = []
    for cix in range(n_cores):
        s = slice(cix * np_per_core, (cix + 1) * np_per_core)
        streams = (
            lo[s].reshape(P, fpl),
            up[s].reshape(P, fpl),
            tb[s].reshape(P, fpl),
            pb[s].reshape(P, fpl),
            sgn[s].reshape(P, fpl),
        )
        # tile-major: per partition, each tile's 5 stream-chunks contiguous
        parts = []
        off = 0
        for fd in widths:
            for st in streams:
                parts.append(st[:, off : off + fd])
            off += fd
        big = np.concatenate(parts, axis=1)
        in_maps.append({"big": np.ascontiguousarray(big), "consts": consts})
    return in_maps


def _combine(core_outs, core_sums, widths=TILE_WIDTHS, n=N):
    """core_outs: [P, 2*n_tiles] ACT accumulators per core (S_sq | S_vd).
    core_sums: [2, MM] column sums per core (S_w | S_d)."""
    n_tiles = len(widths)
    s_sq = s_vd = s_w = s_d = 0.0
    for o, ss in zip(core_outs, core_sums):
        o = np.asarray(o, dtype=np.float64)
        ss = np.asarray(ss, dtype=np.float64)
        s_sq += o[:, 0:n_tiles].sum()
        s_vd += o[:, n_tiles : 2 * n_tiles].sum()
        s_w += ss[0].sum()
        s_d += ss[1].sum()

    center_loss = s_sq / n
    width_loss = -s_w / n                  # sum(up - lo) = -sum(lo - up)
    valid_penalty = s_vd / n
    direction_penalty = s_d
    total = (
        center_loss * 10.0
        + 0.1 * width_loss
        + 10.0 * valid_penalty
        + 0.5 * direction_penalty / n
    )
    return np.float32(total)


def _run(inputs, trace=False):
    """Run the SPMD kernel; returns (scalar_result, BassKernelResults)."""
    from concourse.bass_utils import run_bass_kernel_spmd

    nc = _get_nc()
    in_maps = _shard(inputs)
    res = run_bass_kernel_spmd(
        nc, in_maps, core_ids=list(range(N_CORES)), trace=trace
    )
    core_outs = [res.results[c]["out"] for c in range(N_CORES)]
    core_sums = [res.results[c]["sums"] for c in range(N_CORES)]
    return _combine(core_outs, core_sums), res


def kernel(**inputs) -> np.ndarray:
    result, _ = _run(inputs, trace=False)
    return result
